# revision 22
# baseline (speedup 1.0000x reference)
"""Trainium2 Bass kernel for nn_CAM_62852551409742.

Math (reference):
  f = feats[:, :, 0, :]                               [R,B,T], R=4, B=512, T=150
  feat_n = feats.reshape(B, R*T)                      [B,K], K=600
  att[r,b,t,k] = tanh(a[r]*f[r,b,t] * feat_n[b,k])
  Hm = relu(att @ Wc[r].T + f*W[r])                   [R,B,T,32]
  attf = Hm @ Wh[r] + f                               [R,B,T]
  ff[b, r*T+t] = attf[r,b,t]
  out = (ff @ W1.T + b1) @ W2.T + b2                  [B,1,7]

Strategy: data-parallel over B across 8 cores (64 batches each). On device,
per 8-batch group: DVE builds z[k,(b,r,t)] = af broadcast * fn column
(tensor_scalar, 4x bf16), ACT applies tanh in place with huge free dims,
PE contracts k against Wc^T tiles into PSUM [(r,c) x (b,t)] chunks
(f*W folded in as an extra contraction row on the last k-tile), DVE relu ->
Hm_all bf16. Final: the linear tail is algebraically collapsed on the host
(Wx = W2@W1, U[(r,c),t,i] = Wh[r,c]*Wx[i,r*T+t]) so 150 small matmuls
(lhsT = Hm slice, rhs = U_t) plus 5 fp32 matmuls (f^T x Wx^T) accumulate the
final [64,7] directly in PSUM.
"""

from contextlib import ExitStack

import numpy as np
import ml_dtypes

import concourse.bacc as bacc
import concourse.bass as bass
import concourse.tile as tile
from concourse import mybir
from concourse import bass_utils

R, B, T, H = 4, 512, 150, 32
K = R * T                      # 600
NCORES = 8
BL = B // NCORES               # 64 batches per core
G, GB = 8, 8                   # 8 groups of 8 batches
KTS = [(0, 128), (128, 128), (256, 128), (384, 128), (512, 88)]
F32 = mybir.dt.float32
BF16 = mybir.dt.bfloat16
BF = ml_dtypes.bfloat16

_CACHE = {}


def build_nc():
    nc = bacc.Bacc("TRN2", target_bir_lowering=False)
    af_d = nc.dram_tensor("af", [BL, K], BF16, kind="ExternalInput")
    f_d = nc.dram_tensor("fr", [1, BL, K], BF16, kind="ExternalInput")
    fn_d = nc.dram_tensor("fn", [128, 5, BL], F32, kind="ExternalInput")
    wc_d = nc.dram_tensor("wc", [128, R, 5, H], BF16, kind="ExternalInput")
    u_d = nc.dram_tensor("u", [128, T, 7], BF16, kind="ExternalInput")
    ft_d = nc.dram_tensor("ft", [128, 5, BL], F32, kind="ExternalInput")
    wx_d = nc.dram_tensor("wx", [128, 5, 7], F32, kind="ExternalInput")
    bx_d = nc.dram_tensor("bx", [7, 1], F32, kind="ExternalInput")
    out_d = nc.dram_tensor("out", [7, BL], F32, kind="ExternalOutput")

    with tile.TileContext(nc) as tc, ExitStack() as ctx:
        consts = ctx.enter_context(tc.tile_pool(name="consts", bufs=1))
        attp = ctx.enter_context(tc.tile_pool(name="att", bufs=2))
        afp = ctx.enter_context(tc.tile_pool(name="afp", bufs=2))
        hmp = ctx.enter_context(tc.tile_pool(name="hm", bufs=1))
        outp = ctx.enter_context(tc.tile_pool(name="outp", bufs=1))
        psum = ctx.enter_context(tc.tile_pool(name="ps", bufs=6, space="PSUM"))
        psum_o = ctx.enter_context(tc.tile_pool(name="pso", bufs=1, space="PSUM"))

        # startup-critical loads first: fn (z-pass scalars, kt0 first), then
        # group 0's af broadcasts; bulk constants stream in behind them.
        fn_sb = consts.tile([128, 5, BL], F32)
        for kt in range(5):
            nc.sync.dma_start(out=fn_sb[:, kt, :], in_=fn_d[:, kt, :])
        wc_sb = consts.tile([128, R, 5, H], BF16)
        u_sb = consts.tile([128, T, 7], BF16)
        ft_sb = consts.tile([128, 5, BL], F32)
        wx_sb = consts.tile([128, 5, 7], F32)
        bx_sb = consts.tile([7, 1], F32)
        hm_all = hmp.tile([128, BL * T], BF16)

        # variable group sizes: tiny leading groups start the ACT pipeline
        # early (head latency is af-broadcast bound).
        SZ = [1, 3, 4, 8, 8, 8, 8, 8, 8, 8]
        assert sum(SZ) == BL
        cum = 0
        op = None
        for g, nb_g in enumerate(SZ):
            b0 = cum
            cum += nb_g
            af_g = afp.tile([128, GB, K], BF16, tag="afg")
            for b in range(nb_g):
                # early batches gate the ACT pipeline start: split their
                # partition-broadcasts across queues for transfer parallelism
                nsplit = 4 if b0 + b == 0 else (2 if b0 + b < 4 else 1)
                step = 128 // nsplit
                for ci in range(nsplit):
                    eng = nc.sync if ci % 2 == 0 else nc.gpsimd
                    eng.dma_start(
                        out=af_g[ci * step : (ci + 1) * step, b, :],
                        in_=bass.AP(
                            tensor=af_d,
                            offset=(b0 + b) * K,
                            ap=[[0, step], [1, K]],
                        ),
                    )
            if g == 0:
                nc.scalar.dma_start(out=wc_sb[:], in_=wc_d[:])
            if g == 2:
                nc.sync.dma_start(out=u_sb[:], in_=u_d[:])
                nc.sync.dma_start(out=ft_sb[:], in_=ft_d[:])
                nc.sync.dma_start(out=wx_sb[:], in_=wx_d[:])
                nc.sync.dma_start(out=bx_sb[:], in_=bx_d[:])
            atts = []
            for kt, (k0, kp) in enumerate(KTS):
                at = attp.tile([128, GB, K], BF16, tag=f"att{kt}")
                atts.append(at)
                if kt == 4:
                    nc.sync.dma_start(
                        out=at[88:89, 0:nb_g, :], in_=f_d[0:1, b0 : b0 + nb_g, :]
                    )
                for b in range(nb_g):
                    nc.vector.tensor_scalar_mul(
                        out=at[0:kp, b, :],
                        in0=af_g[0:kp, b, :],
                        scalar1=fn_sb[0:kp, kt, b0 + b : b0 + b + 1],
                    )
                nc.scalar.activation(
                    out=at[0:kp, 0:nb_g, :],
                    in_=at[0:kp, 0:nb_g, :],
                    func=mybir.ActivationFunctionType.Tanh,
                )
            chunks = [(s, min(3, nb_g - s)) for s in range(0, nb_g, 3)]
            ptiles = []
            for ci, (_, nb) in enumerate(chunks):
                pt = psum.tile([128, nb * T], F32, tag="hmps", padded_shape=[None, 512])
                ptiles.append(pt)
            for kt, (k0, kp) in enumerate(KTS):
                pp = kp + 1 if kt == 4 else kp
                for r in range(R):
                    lhsT = wc_sb[0:pp, r, kt, :]
                    for ci, (s, nb) in enumerate(chunks):
                        nc.tensor.matmul(
                            out=ptiles[ci][r * H : (r + 1) * H, 0 : nb * T],
                            lhsT=lhsT,
                            rhs=atts[kt][0:pp, s : s + nb, r * T : (r + 1) * T],
                            start=(kt == 0),
                            stop=(kt == 4),
                            tile_position=(0, r * H),
                            skip_group_check=True,
                        )
            for ci, (s, nb) in enumerate(chunks):
                nc.vector.tensor_scalar_max(
                    out=hm_all[:, (b0 + s) * T : (b0 + s + nb) * T],
                    in0=ptiles[ci][:, 0 : nb * T],
                    scalar1=0.0,
                )
            # final-output accumulation in two b-halves: the first half's 155
            # matmuls run while ACT is still busy with later groups.
            if cum == BL // 2 or cum == BL:
                hm3 = hm_all.rearrange("p (b t) -> p b t", t=T)
                h0 = 0 if cum == BL // 2 else BL // 2
                if op is None:
                    op = psum_o.tile([7, BL], F32, padded_shape=[None, 512])
                for t in range(T):
                    nc.tensor.matmul(
                        out=op[:, h0 : h0 + BL // 2],
                        lhsT=u_sb[:, t, :],
                        rhs=hm3[:, h0 : h0 + BL // 2, t],
                        start=(t == 0),
                        stop=False,
                    )
                for kt, (k0, kp) in enumerate(KTS):
                    nc.tensor.matmul(
                        out=op[:, h0 : h0 + BL // 2],
                        lhsT=wx_sb[0:kp, kt, :],
                        rhs=ft_sb[0:kp, kt, h0 : h0 + BL // 2],
                        start=False,
                        stop=(kt == 4),
                    )

        ob = outp.tile([7, BL], F32)
        nc.vector.tensor_scalar_add(out=ob[:], in0=op[:], scalar1=bx_sb[:])
        nc.sync.dma_start(out=out_d[:], in_=ob[:])

    nc.finalize()
    return nc


def _host_prep(feats, a, W, Wc, Wh, W1, b1, W2, b2):
    """Per-core input maps. feats: [R,B,1,T] fp32."""
    f = feats[:, :, 0, :]                              # [R,B,T]
    af_full = a[:, None, None] * f                     # [R,B,T]
    feat_n = feats.reshape(B, K)                       # [B,K]
    Wx = W2 @ W1                                       # [7,K]
    bx = W2 @ b1 + b2                                  # [7]

    # U[(r,c), t, i] = Wh[r,c] * Wx[i, r*T+t]
    U = np.zeros((128, T, 7), np.float32)
    for r in range(R):
        blk = Wx[:, r * T : (r + 1) * T].T             # [T,7]
        U[r * H : (r + 1) * H] = Wh[r][:, None, None] * blk[None]

    # wc_h[p, r, kt, c]: Wc[r].T rows per k-tile; kt4 row 88 = W[r]
    wc_h = np.zeros((128, R, 5, H), np.float32)
    for r in range(R):
        for kt, (k0, kp) in enumerate(KTS):
            wc_h[:kp, r, kt, :] = Wc[r, :, k0 : k0 + kp].T
        wc_h[88, r, 4, :] = W[r]

    wx_h = np.zeros((128, 5, 7), np.float32)
    for kt, (k0, kp) in enumerate(KTS):
        wx_h[:kp, kt, :] = Wx[:, k0 : k0 + kp].T

    fT_full = np.concatenate([f[r].T for r in range(R)], axis=0)  # [K, B]

    in_maps = []
    for m in range(NCORES):
        b0 = m * BL
        af_h = np.ascontiguousarray(
            af_full[:, b0 : b0 + BL, :].transpose(1, 0, 2).reshape(BL, K)
        ).astype(BF)
        f_h = np.ascontiguousarray(
            f[:, b0 : b0 + BL, :].transpose(1, 0, 2).reshape(1, BL, K)
        ).astype(BF)
        fn_h = np.zeros((128, 5, BL), np.float32)
        for kt, (k0, kp) in enumerate(KTS):
            fn_h[:kp, kt, :] = feat_n[b0 : b0 + BL, k0 : k0 + kp].T
        ft_h = np.zeros((128, 5, BL), np.float32)
        for kt, (k0, kp) in enumerate(KTS):
            ft_h[:kp, kt, :] = fT_full[k0 : k0 + kp, b0 : b0 + BL]
        in_maps.append(
            {
                "af": af_h,
                "fr": f_h,
                "fn": fn_h,
                "wc": wc_h.astype(BF),
                "u": U.astype(BF),
                "ft": ft_h,
                "wx": wx_h,
                "bx": bx.astype(np.float32).reshape(7, 1),
            }
        )
    return in_maps


def kernel(feats_list, a, W, Wc, Wh, W1, b1, W2, b2):
    feats = np.asarray(feats_list, np.float32)
    in_maps = _host_prep(
        feats,
        np.asarray(a, np.float32),
        np.asarray(W, np.float32),
        np.asarray(Wc, np.float32),
        np.asarray(Wh, np.float32),
        np.asarray(W1, np.float32),
        np.asarray(b1, np.float32),
        np.asarray(W2, np.float32),
        np.asarray(b2, np.float32),
    )
    if "nc" not in _CACHE:
        _CACHE["nc"] = build_nc()
    res = bass_utils.run_bass_kernel_spmd(
        _CACHE["nc"], in_maps, core_ids=list(range(NCORES))
    )
    _CACHE["last_result"] = res
    out = np.concatenate([r["out"].T for r in res.results], axis=0)  # [B,7]
    return out[:, None, :].astype(np.float32)                        # [B,1,7]
